# revision 3
# baseline (speedup 1.0000x reference)
"""Trainium2 Bass kernel for 16-head causal MultiHeadAttention — v4.

bf16 compute (fp32 PSUM) + packed DRAM layouts for DMA line efficiency +
dual HWDGE queues.

Measured HW facts driving this version (see memory):
- Per-core DMA runs ~60-65 GB/s for <=4KB per-partition lines but ~160 GB/s
  at 8KB lines, and the SP(sync) + ACT(scalar) HWDGE queues run in parallel
  (2-queue split measured 4.8x faster than single-queue).  So all bulk
  tensors are host-packed into [128, N] row-major layouts giving 8KB
  contiguous lines, loaded as [128, 4096] (1MB) DMAs alternated across both
  queues.
- Matmuls run at full rate (209ns per 512-col bf16 matmul incl. weight
  load), so the schedule keeps the in-order PE fed with "filler" projection
  matmuls (K chunks 1-3, V blocks 4-15, Q of chunk c+1, out-proj of chunks
  0-2) pumped between the ACT(exp)-paced attention blocks.

Sharding (8 cores): core c handles batch n = c//2 and head group g = c%2
(8 heads each).  Host sums the two partial output projections per batch and
adds bo.

Packed layouts (p is the SBUF partition index):
  x:   xp[p, h*8192 + dh*4096 + dq*1024 + t'] = x[1024h + t', 128*(4dh+dq) + p]
  wq/wk: wp[p, d*512 + j]  = w[128d + p, j]         (j in [0,512))
  wv:    same as wq/wk
  wo:    wop[p, g*1024 + e] = wo[128g + p, e]       (e in [0,1024))
  out:   outp[p, b*1024 + e] = out[128b + p, e]     (b = tq block index)

PSUM (8 banks): psS pair-tiles 2x[128,1024] (4) + psA 2x[65,512] (2) +
filler/projection accumulators 2x[128,512] (2).
"""

import os
import sys

import numpy as np

for _p in ("/opt/trn_rl_repo",):
    if _p not in sys.path and os.path.isdir(_p):
        sys.path.insert(0, _p)

import concourse.bacc as bacc
import concourse.mybir as mybir
import concourse.tile as tile
from concourse.bass_utils import run_bass_kernel_spmd

D_M = 1024
HEADS = 16
D_K = 64
N_B = 4
T = 2048
N_CORES = 8
HPC = HEADS // 2          # heads per core = 8
J = HPC * D_K             # per-core projection width = 512
G = J // 128              # j-tiles per core = 4
DT = D_M // 128           # d (contraction) tiles = 8
CHUNK = 512               # tq chunk (free dim of most matmuls)
NCHUNK = T // CHUNK       # 4
NBLK = T // 128           # tk blocks = 16
XW = 4096                 # packed x DMA width ([128, 4096] bf16 = 8KB lines)
F32 = mybir.dt.float32
BF16 = mybir.dt.bfloat16
NEG = -1.0e30

_cached_nc = {}


def build_nc(loop_n=None, phases=('kv', 'q', 'attn', 'out')):
    """loop_n: if set, weights/consts load once outside a HW For_i loop and
    the x-load + compute + store body repeats loop_n times (timing variant;
    numerically identical every iteration)."""
    nc = bacc.Bacc(None)

    xq = nc.declare_dram_parameter("xqp", [128, 16384], BF16, isOutput=False)
    xk = nc.declare_dram_parameter("xkp", [128, 16384], BF16, isOutput=False)
    xv = nc.declare_dram_parameter("xvp", [128, 16384], BF16, isOutput=False)
    wq = nc.declare_dram_parameter("wqp", [128, 4096], BF16, isOutput=False)
    wk = nc.declare_dram_parameter("wkp", [128, 4096], BF16, isOutput=False)
    wv = nc.declare_dram_parameter("wvp", [128, 4096], BF16, isOutput=False)
    wo = nc.declare_dram_parameter("wop", [128, 4096], BF16, isOutput=False)
    bq = nc.declare_dram_parameter("bq2", [128, G], F32, isOutput=False)
    bk = nc.declare_dram_parameter("bk2", [128, G], F32, isOutput=False)
    bvb = nc.declare_dram_parameter("bvb", [128, J], F32, isOutput=False)
    padb = nc.declare_dram_parameter("padb", [128, NBLK], F32, isOutput=False)
    trim = nc.declare_dram_parameter("trimask", [128, 128], F32, isOutput=False)
    out = nc.declare_dram_parameter("outp", [128, 16384], BF16, isOutput=True)

    Exp = mybir.ActivationFunctionType.Exp
    Ident = mybir.ActivationFunctionType.Identity

    with tile.TileContext(nc) as tc:
        with (
            tc.tile_pool(name="consts", bufs=1) as cpool,
            tc.tile_pool(name="wproj", bufs=1) as wpool,
            tc.tile_pool(name="persist", bufs=1) as ppool,
            tc.tile_pool(name="xk", bufs=4) as xkpool,
            tc.tile_pool(name="xv", bufs=4) as xvpool,
            tc.tile_pool(name="xq", bufs=4) as xqpool,
            tc.tile_pool(name="qht", bufs=8) as qpool,
            tc.tile_pool(name="ant", bufs=12) as apool,
            tc.tile_pool(name="exps", bufs=4) as epool,
            tc.tile_pool(name="rec", bufs=2) as rpool,
            tc.tile_pool(name="osb", bufs=1) as opool,
            tc.tile_pool(name="ps_s", bufs=2, space="PSUM") as psum_s,
            tc.tile_pool(name="ps_f", bufs=2, space="PSUM") as psum_f,
            tc.tile_pool(name="ps_a", bufs=2, space="PSUM") as psum_ap,
        ):
            qtog = [0]

            def dma2(out_ap, in_ap):
                """Alternate DMAs across the SP and ACT HWDGE queues."""
                eng = nc.sync if qtog[0] % 2 == 0 else nc.scalar
                qtog[0] += 1
                eng.dma_start(out=out_ap, in_=in_ap)

            def load_x_h(xdram, label, pool, h):
                """2 tiles of [128, 4096] for T-half h (d-halves 0, 1)."""
                tiles = []
                for dh in range(2):
                    t_ = pool.tile([128, XW], BF16, name=f"{label}{h}{dh}", tag=label)
                    dma2(t_[:, :], xdram[:, (2 * h + dh) * XW:(2 * h + dh + 1) * XW])
                    tiles.append(t_)
                return tiles

            def xs(tiles, d, c):
                """[128, 512] slice of packed x for (d-tile, chunk)."""
                h, cc = divmod(c, 2)
                dh, dq = divmod(d, 4)
                return tiles[h][dh][:, dq * 1024 + cc * CHUNK:dq * 1024 + (cc + 1) * CHUNK]

            def xsb(tiles, d, i):
                """[128, 128] slice of packed x for (d-tile, tk block i)."""
                h, tl = divmod(i, 8)
                dh, dq = divmod(d, 4)
                return tiles[h][dh][:, dq * 1024 + tl * 128:dq * 1024 + (tl + 1) * 128]

            def load_w(dram, label):
                t_ = wpool.tile([128, 4096], BF16, name=label, tag=label)
                dma2(t_[:, :], dram[:, :])
                return t_

            def load_consts():
                trim_t = cpool.tile([128, 128], F32, name="trim_t", tag="trim")
                dma2(trim_t[:, :], trim[:, :])
                padb_t = cpool.tile([128, NBLK], F32, name="padb_t", tag="padb")
                dma2(padb_t[:, :], padb[:, :])
                bq_t = cpool.tile([128, G], F32, name="bq_t", tag="bq")
                dma2(bq_t[:, :], bq[:, :])
                bvb_t = cpool.tile([128, J], F32, name="bvb_t", tag="bvb")
                dma2(bvb_t[:, :], bvb[:, :])
                return trim_t, padb_t, bq_t, bvb_t

            # Persistent activations.
            khT = []  # khT[g]: [128, T] bf16 — heads 2g (rows 0-63), 2g+1 (64-127)
            for g in range(G):
                khT.append(ppool.tile([128, T], BF16, name=f"khT{g}", tag=f"khT{g}"))
            vh = []   # vh[i]: [128, 520] bf16 — per head h: cols 65h..+63 = v, 65h+64 = 1
            for i in range(NBLK):
                vh.append(ppool.tile([128, 520], BF16, name=f"vh{i}", tag=f"vh{i}"))

            if loop_n is not None:
                bk_t = cpool.tile([128, G], F32, name="bk_t", tag="bk")
                dma2(bk_t[:, :], bk[:, :])
                preloaded = (load_consts(), bk_t, load_w(wk, "wk"),
                             load_w(wq, "wq"), load_w(wv, "wv"), load_w(wo, "wo"))
            else:
                preloaded = None

            def body():
                # ---- DMA queue: first-use order over both queues ----------
                if preloaded is None:
                    wk_t = load_w(wk, "wk")
                    xk_h0 = load_x_h(xk, "xk", xkpool, 0)
                    bk_t = cpool.tile([128, G], F32, name="bk_t", tag="bk")
                    dma2(bk_t[:, :], bk[:, :])
                    trim_t, padb_t, bq_t, bvb_t = load_consts()
                    wv_t = load_w(wv, "wv")
                    xv_h0 = load_x_h(xv, "xv", xvpool, 0)
                    wq_t = load_w(wq, "wq")
                    xq_h0 = load_x_h(xq, "xq", xqpool, 0)
                    xk_t = [xk_h0, load_x_h(xk, "xk", xkpool, 1)]
                    xv_t = [xv_h0, load_x_h(xv, "xv", xvpool, 1)]
                    xq_t = [xq_h0, load_x_h(xq, "xq", xqpool, 1)]
                    wo_t = load_w(wo, "wo")
                else:
                    (trim_t, padb_t, bq_t, bvb_t), bk_t, wk_t, wq_t, wv_t, wo_t = preloaded
                    xk_h0 = load_x_h(xk, "xk", xkpool, 0)
                    xv_h0 = load_x_h(xv, "xv", xvpool, 0)
                    xq_h0 = load_x_h(xq, "xq", xqpool, 0)
                    xk_t = [xk_h0, load_x_h(xk, "xk", xkpool, 1)]
                    xv_t = [xv_h0, load_x_h(xv, "xv", xvpool, 1)]
                    xq_t = [xq_h0, load_x_h(xq, "xq", xqpool, 1)]

                def wqk(w_t, d, g):
                    return w_t[:, d * 512 + g * 128:d * 512 + (g + 1) * 128]

                # ---- K projection: chunk 0 (d-outer, consumed as DMA lands)
                if 'kv' not in phases:
                    for g in range(G):
                        nc.gpsimd.memset(khT[g][:, :].bitcast(mybir.dt.uint16), 0)
                    for i in range(NBLK):
                        nc.gpsimd.memset(vh[i][:, :].bitcast(mybir.dt.uint16), 0x3F80)
                kps = [psum_s.tile([128, 2 * CHUNK], F32, name=f"psK0p{i}", tag="ps")
                       for i in range(2)] if 'kv' in phases else []
                for d in range(DT if 'kv' in phases else 0):
                    for g in range(G):
                        nc.tensor.matmul(
                            kps[g // 2][:, (g % 2) * CHUNK:(g % 2 + 1) * CHUNK],
                            wqk(wk_t, d, g), xs(xk_t, d, 0),
                            start=(d == 0), stop=(d == DT - 1),
                            skip_group_check=True,
                        )
                for g in range(G if 'kv' in phases else 0):
                    nc.scalar.activation(
                        khT[g][:, 0:CHUNK],
                        kps[g // 2][:, (g % 2) * CHUNK:(g % 2 + 1) * CHUNK],
                        Ident, bias=bk_t[:, g:g + 1], scale=1.0,
                    )

                # ---- filler generators ------------------------------------
                def k_unit(c, g):
                    """K-projection for chunk c >= 1, head-pair g."""
                    if 'kv' not in phases:
                        return
                    ps = psum_f.tile([128, CHUNK], F32, name=f"psK{c}_{g}", tag="pf")
                    for d in range(DT):
                        nc.tensor.matmul(
                            ps[:, :], wqk(wk_t, d, g), xs(xk_t, d, c),
                            start=(d == 0), stop=(d == DT - 1),
                        )
                        yield
                    nc.scalar.activation(
                        khT[g][:, c * CHUNK:(c + 1) * CHUNK], ps[:, :],
                        Ident, bias=bk_t[:, g:g + 1], scale=1.0,
                    )

                def v_unit(i):
                    """V-projection for tk block i; one matmul per yield."""
                    if 'kv' not in phases:
                        return
                    ps = psum_f.tile([128, J], F32, name=f"psV{i}", tag="pf")
                    for d in range(DT):
                        nc.tensor.matmul(
                            ps[:, :], xsb(xv_t, d, i), wv_t[:, d * 512:(d + 1) * 512],
                            start=(d == 0), stop=(d == DT - 1),
                        )
                        yield
                    dst = vh[i][:, 0:J + HPC].rearrange("p (h e) -> p h e", e=65)
                    nc.vector.tensor_add(
                        dst[:, :, 0:64],
                        ps[:, :].rearrange("p (h e) -> p h e", e=64),
                        bvb_t[:, :].rearrange("p (h e) -> p h e", e=64),
                    )
                    nc.gpsimd.memset(dst[:, :, 64:65].bitcast(mybir.dt.uint16), 0x3F80)

                def q_unit(c, g, sink):
                    qt = qpool.tile([128, CHUNK], BF16, name=f"qht{c}_{g}", tag="qht")
                    if 'q' not in phases:
                        nc.gpsimd.memset(qt[:, :].bitcast(mybir.dt.uint16), 0)
                        sink[g] = qt
                        return
                    ps = psum_f.tile([128, CHUNK], F32, name=f"psQ{c}_{g}", tag="pf")
                    for d in range(DT):
                        nc.tensor.matmul(
                            ps[:, :], wqk(wq_t, d, g), xs(xq_t, d, c),
                            start=(d == 0), stop=(d == DT - 1),
                        )
                        yield
                    if c == 0:
                        nc.scalar.activation(qt[:, :], ps[:, :], Ident,
                                             bias=bq_t[:, g:g + 1], scale=1.0)
                    else:
                        nc.vector.tensor_scalar_add(qt[:, :], ps[:, :], bq_t[:, g:g + 1])
                    sink[g] = qt

                def o_unit(c, mt, e, ant_c, ob, final=False):
                    """Out-projection for (chunk c, tq block mt, e-half).
                    Writes into the chunk's shared [128, 4096] ob tile; the
                    (mt=3, e=1) unit also emits the chunk's two store DMAs."""
                    if final and (mt + e) % 2 == 0:
                        ps = psum_s.tile([128, CHUNK], F32, name=f"psO{c}_{mt}_{e}",
                                         tag="ps", padded_shape=[128, 2 * CHUNK])
                    else:
                        ps = psum_f.tile([128, CHUNK], F32, name=f"psO{c}_{mt}_{e}", tag="pf")
                    for g in range(G):
                        nc.tensor.matmul(
                            ps[:, :],
                            ant_c[g][:, mt * 128:(mt + 1) * 128],
                            wo_t[:, g * 1024 + e * CHUNK:g * 1024 + (e + 1) * CHUNK],
                            start=(g == 0), stop=(g == G - 1),
                        )
                        yield
                    nc.vector.tensor_copy(ob[:, mt * 1024 + e * CHUNK:
                                             mt * 1024 + (e + 1) * CHUNK], ps[:, :])
                    if mt == 3 and e == 1:
                        nc.sync.dma_start(
                            out=out[:, c * 4096:c * 4096 + 2048], in_=ob[:, 0:2048])
                        nc.scalar.dma_start(
                            out=out[:, c * 4096 + 2048:(c + 1) * 4096], in_=ob[:, 2048:4096])

                class Pump:
                    """Sequential emitter over filler generators."""

                    def __init__(self, gens):
                        self.gens = list(gens)

                    def pump(self, n):
                        done = 0
                        while self.gens and done < n:
                            try:
                                next(self.gens[0])
                                done += 1
                            except StopIteration:
                                self.gens.pop(0)
                        return done

                    def drain(self):
                        while self.gens:
                            self.pump(1 << 30)

                # ---- attention for one (chunk, head-pair g) ---------------
                def attn_group(c, g, qt, at, pump_blk):
                    if 'attn' not in phases:
                        nc.gpsimd.memset(at[:, :].bitcast(mybir.dt.uint16), 0)
                        return
                    nb = 4 * c + 4  # causal: tk blocks 0..nb-1
                    ps_a = [psum_ap.tile([65, CHUNK], F32,
                                         name=f"psA{c}_{2 * g + hh}", tag="pa")
                            for hh in range(2)]
                    # 2-deep software pipeline: A(bk-2) is emitted after S(bk).
                    pend = []  # [(cs, es_pair), ...] oldest first

                    def emit_a(bk_, first):
                        pcs, pes = pend.pop(0)
                        for hh in range(2):
                            nc.tensor.matmul(
                                ps_a[hh][:, pcs:],
                                vh[bk_][:, 65 * (2 * g + hh):65 * (2 * g + hh) + 65],
                                pes[:, 512 * hh + pcs:512 * hh + 512],
                                start=first, stop=(bk_ == nb - 1),
                            )

                    for bk in range(nb):
                        m = bk - 4 * c  # >=0 on the diagonal superblock
                        cs = 128 * m if m >= 0 else 0  # exact causal start
                        ps_s = psum_s.tile([128, 2 * CHUNK], F32,
                                           name=f"psS{c}_{g}_{bk}", tag="ps")
                        for hh in range(2):
                            nc.tensor.matmul(
                                ps_s[:, 512 * hh + cs:512 * hh + 512],
                                khT[g][hh * 64:(hh + 1) * 64, bk * 128:(bk + 1) * 128],
                                qt[hh * 64:(hh + 1) * 64, cs:],
                                start=True, stop=True,
                            )
                        if m >= 0:
                            for hh in range(2):
                                nc.vector.tensor_add(
                                    ps_s[:, 512 * hh + cs:512 * hh + cs + 128],
                                    ps_s[:, 512 * hh + cs:512 * hh + cs + 128],
                                    trim_t[:, :],
                                )
                        if bk >= 2:
                            emit_a(bk - 2, first=(bk == 2))
                        es = epool.tile([128, 2 * CHUNK], BF16,
                                        name=f"es{c}_{g}_{bk}", tag="es")
                        if cs > 0:
                            nc.scalar.activation(
                                es[:, 0:2 * CHUNK].rearrange(
                                    "p (h e) -> p h e", h=2)[:, :, cs:],
                                ps_s[:, 0:2 * CHUNK].rearrange(
                                    "p (h e) -> p h e", h=2)[:, :, cs:],
                                Exp, bias=padb_t[:, bk:bk + 1], scale=0.125,
                            )
                        else:
                            nc.scalar.activation(
                                es[:, :], ps_s[:, :], Exp,
                                bias=padb_t[:, bk:bk + 1], scale=0.125,
                            )
                        pend.append((cs, es))
                        pump_blk()
                    emit_a(nb - 2, first=(nb == 2))
                    emit_a(nb - 1, first=False)
                    pump_blk(extra=10)
                    for hh in range(2):
                        h = 2 * g + hh
                        # normalize: rows 0-63 = A^T numerator, row 64 = denom.
                        # partition_broadcast reads partition 0 of the
                        # underlying tile, so the reciprocal lands there.
                        rc = rpool.tile([128, CHUNK], F32, name=f"rc{c}_{h}", tag="rc")
                        nc.vector.reciprocal(rc[0:1, :], ps_a[hh][64:65, :])
                        rb = rpool.tile([128, CHUNK], F32, name=f"rb{c}_{h}", tag="rb")
                        nc.gpsimd.partition_broadcast(rb[0:64, :], rc[0:1, :])
                        nc.vector.tensor_mul(
                            at[hh * 64:(hh + 1) * 64, :], ps_a[hh][0:64, :], rb[0:64, :],
                        )

                # ---- V blocks 0-3, Q chunk 0 (pre-attention) --------------
                Pump([v_unit(i) for i in range(4)]).drain()
                qht_c = [None] * G
                Pump([q_unit(0, g, qht_c) for g in range(G)]).drain()

                # ---- chunk loop with interleaved fillers ------------------
                ants = []
                for c in range(NCHUNK):
                    ant_c = [apool.tile([128, CHUNK], BF16, name=f"ant{c}_{g}", tag="ant")
                             for g in range(G)]
                    qht_next = [None] * G
                    cats = []
                    if c < NCHUNK - 1:
                        cats.append([k_unit(c + 1, g) for g in range(G)])
                        cats.append([v_unit(4 * c + 4 + i) for i in range(4)])
                        cats.append([q_unit(c + 1, g, qht_next) for g in range(G)])
                    gens = []
                    # out-proj of chunks 0-2 is deferred to the late
                    # (filler-starved) chunks
                    o_sched = {} if 'out' not in phases else \
                              {2: [(0, ants[0])] if len(ants) > 0 else [],
                               3: [(oc, ants[oc]) for oc in range(1, len(ants))]}
                    # categories run sequentially: K-unit ACT epilogues must
                    # complete before the exp queue deepens, and the 1-buf ob
                    # tile serializes out-proj chunks
                    for cat in cats:
                        gens += cat
                    for (oc, oant) in o_sched.get(c, []):
                        ob = opool.tile([128, 4096], BF16, name=f"ob{oc}", tag="ob")
                        gens += [o_unit(oc, mt, e, oant, ob)
                                 for mt in range(4) for e in range(2)]
                    pump = Pump(gens)
                    nsteps = (8 * (3 * G) if c < NCHUNK - 1 else 0) \
                        + 8 * G * len(o_sched.get(c, []))
                    nblocks = G * (4 * c + 4)
                    state = {"blk": 0, "emitted": 0}

                    def pump_blk(extra=0):
                        state["blk"] += 1
                        target = min(nsteps, nsteps * state["blk"] // nblocks + extra)
                        state["emitted"] += pump.pump(target - state["emitted"])

                    pump_blk(extra=4)
                    for g in range(G):
                        attn_group(c, g, qht_c[g], ant_c[g], pump_blk)
                    pump.drain()
                    ants.append(ant_c)
                    qht_c = qht_next
                # final chunk's out-projection
                if 'out' in phases:
                    ob3 = opool.tile([128, 4096], BF16, name="ob3", tag="ob")
                    Pump([o_unit(NCHUNK - 1, mt, e, ants[3], ob3, final=True)
                          for mt in range(4) for e in range(2)]).drain()

            if loop_n is not None:
                with tc.For_i(0, loop_n, 1):
                    body()
            else:
                body()

    nc.finalize()
    return nc


def get_nc(loop_n=None):
    if loop_n not in _cached_nc:
        _cached_nc[loop_n] = build_nc(loop_n)
    return _cached_nc[loop_n]


def _bf16(a):
    import ml_dtypes
    return np.asarray(a, np.float32).astype(ml_dtypes.bfloat16)


def _pack_x(x):
    """x [T, D_M] fp32 -> [128, 16384] bf16 per the packed layout."""
    a = _bf16(x).T.reshape(2, 4, 128, 2, 1024)       # [dh, dq, p, h, t']
    return np.ascontiguousarray(
        a.transpose(2, 3, 0, 1, 4).reshape(128, 16384))

def _pack_w(w):
    """w [D_M, 512] -> [128, 4096]: wp[p, d*512+j] = w[128d+p, j]."""
    return np.ascontiguousarray(
        _bf16(w).reshape(8, 128, 512).transpose(1, 0, 2).reshape(128, 4096))

def _pack_wo(w):
    """wo [512, 1024] -> [128, 4096]: wop[p, g*1024+e] = wo[128g+p, e]."""
    return np.ascontiguousarray(
        _bf16(w).reshape(4, 128, 1024).transpose(1, 0, 2).reshape(128, 4096))


def make_in_maps(q, k, v, pad_mask, Wq, bq, Wk, bk, Wv, bv, Wo, bo):
    """Host-side sharding: core c -> batch c//2, head-group c%2."""
    f = np.float32
    tri = np.where(
        np.arange(128)[None, :] >= np.arange(128)[:, None], 0.0, NEG
    ).astype(f)  # [tk, tq]: allow tq >= tk
    in_maps = []
    xP = {}
    for n in range(N_B):
        xP[n] = (_pack_x(np.asarray(q[n], f)), _pack_x(np.asarray(k[n], f)),
                 _pack_x(np.asarray(v[n], f)))
    for c in range(N_CORES):
        n, grp = divmod(c, 2)
        js = slice(grp * J, (grp + 1) * J)
        pb = np.where(np.asarray(pad_mask[n]) == 0, NEG, 0.0).astype(f)
        in_maps.append({
            "xqp": xP[n][0], "xkp": xP[n][1], "xvp": xP[n][2],
            "wqp": _pack_w(np.asarray(Wq, f)[:, js]),
            "wkp": _pack_w(np.asarray(Wk, f)[:, js]),
            "wvp": _pack_w(np.asarray(Wv, f)[:, js]),
            "wop": _pack_wo(np.asarray(Wo, f)[js, :]),
            "bq2": np.ascontiguousarray(np.asarray(bq, f)[js].reshape(G, 128).T),
            "bk2": np.ascontiguousarray(np.asarray(bk, f)[js].reshape(G, 128).T),
            "bvb": np.broadcast_to(np.asarray(bv, f)[js], (128, J)).copy(),
            "padb": np.ascontiguousarray(pb.reshape(NBLK, 128).T),
            "trimask": tri,
        })
    return in_maps


def kernel(**inputs) -> np.ndarray:
    nc = get_nc()
    in_maps = make_in_maps(**inputs)
    res = run_bass_kernel_spmd(nc, in_maps, list(range(N_CORES))).results
    bo = np.asarray(inputs["bo"], np.float32)
    out = np.empty((N_B, T, D_M), np.float32)
    for n in range(N_B):
        # outp [128, 16384] -> [2048, 1024]
        a = np.asarray(res[2 * n]["outp"], np.float32) \
            + np.asarray(res[2 * n + 1]["outp"], np.float32)
        out[n] = a.reshape(128, 16, 1024).transpose(1, 0, 2).reshape(T, D_M) + bo
    return out
